# revision 25
# baseline (speedup 1.0000x reference)
"""Trainium2 Bass kernel for the blockwise spiking network (nn_Blocks_86096914416140).

Sharding: data-parallel over batch B=32 across 8 NeuronCores (4 batches/core),
all parameters replicated, zero collectives.

Per-core device algorithm (fp32 throughout; channel-on-partition layout:
partition = c % 128, tiles [128, (c_hi=4, b=4, t=32)]):

  block n:   xr   = x_blk + W @ spikes_prev + beta*v_init@t0         [TensorE]
             q    = cnt_prev + (1 - spiked_prev)                      [GpSimd]
             cur  = min(q, 1) * xr          (refractory gate)         [VectorE]
             mem  = scan: s = beta*s + cur   (seg-reset via pattern)  [VectorE]
             fs   = (mem - 1) > thr          thr = a*(b*p^{t+1})      [VectorE]
             cnt  = seg-cumsum(fs)                                    [VectorE]
             spk  = (cnt == 1) * fs          (bf16, exact 0/1)        [VectorE]
             pdec = scan: s = p*s + spk  -> last col = p^{31-t0}      [VectorE]
             a'   = p^32 * a + (1/p) * pdec_last                      [GpSimd]
             thr' = a' * (b*p^{t+1})                                  [GpSimd]
             spikes regrouped to 8-block chunks for wide DMA          [ScalarE]

The recurrent weight streams through the PE as an exact 3-way bf16 split
(w1+w2+w3 == W to ~2^-27; spike operand is exactly representable in bf16, so
every product is exact and PSUM accumulates in fp32) — 4x the fp32 matmul
streaming rate with fp32-equivalent rounding. The fp32 x tile is added via an
identity matmul into the same PSUM accumulation group; v_init (pre-scaled by
beta on GpSimd) is injected into the t=0 columns the same way.

All tables are precomputed on the host in fp32 to match the reference's
rounding. Validated bit-exact (zero spike flips) against the fp32 reference
both in CoreSim and on the 8 hardware NeuronCores.
"""

import numpy as np

B, C, T_LEN, T = 32, 512, 1024, 32
NB = T_LEN // T          # 32 blocks
NCORES = 8
BPC = B // NCORES        # 4 batches per core
CH = C // 128            # 4 channel tiles
FREE = CH * BPC * T      # 512 free elements per tile
GRP = 8                  # blocks per x-load group

_compiled = None


def _build_program():
    import concourse.bass as bass
    import concourse.bacc as bacc
    import concourse.tile as tile
    from concourse import mybir
    from concourse._compat import with_exitstack
    from contextlib import ExitStack

    f32 = mybir.dt.float32
    Alu = mybir.AluOpType

    bf16 = mybir.dt.bfloat16

    nc = bacc.Bacc()
    x_d = nc.declare_dram_parameter("x_sh", [BPC, C, T_LEN], f32, isOutput=False)
    wt_d = nc.declare_dram_parameter("wt", [48, 128, 128], bf16, isOutput=False)
    betat_d = nc.declare_dram_parameter("betat", [128, CH], f32, isOutput=False)
    ident_d = nc.declare_dram_parameter("ident", [128, 128], f32, isOutput=False)
    betaseg_d = nc.declare_dram_parameter("betaseg", [128, FREE], f32, isOutput=False)
    pseg_d = nc.declare_dram_parameter("pseg", [128, FREE], f32, isOutput=False)
    seg01_d = nc.declare_dram_parameter("seg01", [128, FREE], f32, isOutput=False)
    bp1_d = nc.declare_dram_parameter("bp1", [128, FREE], f32, isOutput=False)
    p32_d = nc.declare_dram_parameter("p32t", [128, CH], f32, isOutput=False)
    invp_d = nc.declare_dram_parameter("invpt", [128, CH], f32, isOutput=False)
    out_d = nc.declare_dram_parameter("out", [BPC, C, T_LEN], f32, isOutput=True)

    def dram_block_ap(handle, ci, nblk, nt):
        # [128 part = c_lo, (b, t)] view of dram[b, ci*128:(ci+1)*128, nblk*T:...]
        return bass.AP(
            tensor=handle,
            offset=ci * 128 * T_LEN + nblk * T,
            ap=[[T_LEN, 128], [C * T_LEN, BPC], [1, nt]],
        )

    def flat(ap4):
        return ap4.rearrange("p c b t -> p (c b t)")

    @with_exitstack
    def kern(ctx: ExitStack, tc: tile.TileContext):
        consts = ctx.enter_context(tc.tile_pool(name="consts", bufs=1))
        xpool = ctx.enter_context(tc.tile_pool(name="xpool", bufs=2))
        work = ctx.enter_context(tc.tile_pool(name="work", bufs=2))
        spkp = ctx.enter_context(tc.tile_pool(name="spkp", bufs=3))
        small = ctx.enter_context(tc.tile_pool(name="small", bufs=3))
        psum = ctx.enter_context(tc.tile_pool(name="psum", bufs=2, space="PSUM"))

        dma = nc.sync

        wt_t = consts.tile([128, 48, 128], bf16, tag="wt")
        dma.dma_start(out=wt_t[:], in_=wt_d[:].rearrange("k p m -> p k m"))
        betat_t = consts.tile([128, CH, 1, 1], f32, tag="betat")
        dma.dma_start(out=betat_t[:],
                      in_=betat_d[:].rearrange("p (c u v) -> p c u v", u=1, v=1))
        id_t = consts.tile([128, 128], f32, tag="ident")
        dma.dma_start(out=id_t[:], in_=ident_d[:])
        betaseg_t = consts.tile([128, FREE], f32, tag="betaseg")
        dma.dma_start(out=betaseg_t[:], in_=betaseg_d[:])
        pseg_t = consts.tile([128, FREE], f32, tag="pseg")
        dma.dma_start(out=pseg_t[:], in_=pseg_d[:])
        seg01_t = consts.tile([128, FREE], f32, tag="seg01")
        dma.dma_start(out=seg01_t[:], in_=seg01_d[:])
        bp1_t = consts.tile([128, CH, BPC, T], f32, tag="bp1")
        dma.dma_start(out=bp1_t[:],
                      in_=bp1_d[:].rearrange("p (c b t) -> p c b t", c=CH, b=BPC))
        p32_t = consts.tile([128, CH, 1, 1], f32, tag="p32t")
        dma.dma_start(out=p32_t[:],
                      in_=p32_d[:].rearrange("p (c u v) -> p c u v", u=1, v=1))
        invp_t = consts.tile([128, CH, 1, 1], f32, tag="invpt")
        dma.dma_start(out=invp_t[:],
                      in_=invp_d[:].rearrange("p (c u v) -> p c u v", u=1, v=1))

        thr0_t = consts.tile([128, CH, BPC, T], f32, tag="thr0")
        nc.vector.memset(thr0_t[:], 0.0)

        import os
        for _rep in range(int(os.environ.get("BENCH_REPEAT", "1"))):
            _run_once(nc, tc, locals())

    def _run_once(nc, tc, env):
        consts = env["consts"]; xpool = env["xpool"]; work = env["work"]
        spkp = env["spkp"]; small = env["small"]; psum = env["psum"]
        dma = env["dma"]
        wt_t = env["wt_t"]; betat_t = env["betat_t"]; id_t = env["id_t"]
        betaseg_t = env["betaseg_t"]; pseg_t = env["pseg_t"]
        seg01_t = env["seg01_t"]; bp1_t = env["bp1_t"]
        p32_t = env["p32_t"]; invp_t = env["invp_t"]; thr0_t = env["thr0_t"]

        # persistent state (assigned per block)
        a_t = thr_t = vb_t = ns_t = spk_prev = cnt_prev = None

        x_g = None
        sgrp = None
        for n in range(NB):
            gi, go = divmod(n, GRP)
            if go == 0:
                x_g = xpool.tile([128, CH, BPC, GRP * T], f32, tag="xg")
                for ci in range(CH):
                    dma.dma_start(out=x_g[:, ci],
                                  in_=dram_block_ap(x_d, ci, gi * GRP, GRP * T))
                sgrp = xpool.tile([128, CH, BPC, GRP, T], f32, tag="sgrp")

            cur_t = work.tile([128, CH, BPC, T], f32, tag="cur")
            if n == 0:
                nc.vector.tensor_copy(out=cur_t[:],
                                      in_=x_g[:, :, :, go * T:(go + 1) * T])
                thr_t = thr0_t
            else:
                xr = psum.tile([128, CH, BPC, T], f32, tag="xr")
                nc.tensor.matmul(
                    out=xr[:],
                    lhsT=id_t[:],
                    rhs=x_g[:, :, :, go * T:(go + 1) * T],
                    start=True, stop=False)
                for v in range(3):
                    for ci in range(CH):
                        for cj in range(CH):
                            nc.tensor.matmul(out=xr[:, ci],
                                             lhsT=wt_t[:, v * 16 + cj * CH + ci],
                                             rhs=spk_prev[:, cj],
                                             start=False, stop=False)
                # v_init (pre-scaled by beta on GpSimd) into t=0 columns
                nc.tensor.matmul(out=xr[:, :, :, 0:1], lhsT=id_t[:],
                                 rhs=vb_t.rearrange("p c b u -> p (c b u)"),
                                 start=False, stop=True)
                # refractory gate: cur = min(cnt_prev + (1-mf), 1) * xr
                q_t = work.tile([128, CH, BPC, T], f32, tag="gate")
                nc.gpsimd.tensor_tensor(
                    out=q_t[:], in0=cnt_prev[:],
                    in1=ns_t.broadcast_to([128, CH, BPC, T]), op=Alu.add)
                nc.vector.scalar_tensor_tensor(
                    out=flat(cur_t), in0=flat(q_t), scalar=1.0, in1=flat(xr),
                    op0=Alu.min, op1=Alu.mult)

            mem_t = work.tile([128, CH, BPC, T], f32, tag="mem")
            nc.vector.tensor_tensor_scan(
                out=flat(mem_t), data0=betaseg_t[:], data1=flat(cur_t),
                initial=0.0, op0=Alu.mult, op1=Alu.add)

            fs_t = work.tile([128, CH, BPC, T], f32, tag="fs")
            nc.vector.scalar_tensor_tensor(
                out=flat(fs_t), in0=flat(mem_t), scalar=1.0, in1=flat(thr_t),
                op0=Alu.subtract, op1=Alu.is_gt)

            cnt_t = work.tile([128, CH, BPC, T], f32, tag="cnt")
            nc.vector.tensor_tensor_scan(
                out=flat(cnt_t), data0=seg01_t[:], data1=flat(fs_t),
                initial=0.0, op0=Alu.mult, op1=Alu.add)

            spk_t = spkp.tile([128, CH, BPC, T], bf16, tag="spk")
            nc.vector.scalar_tensor_tensor(
                out=flat(spk_t), in0=flat(cnt_t), scalar=1.0, in1=flat(fs_t),
                op0=Alu.is_equal, op1=Alu.mult)

            nc.scalar.copy(out=sgrp[:, :, :, go], in_=spk_t[:])
            if go == GRP - 1:
                for ci in range(CH):
                    dma.dma_start(
                        out=dram_block_ap(out_d, ci, gi * GRP, GRP * T),
                        in_=sgrp[:, ci])

            if n < NB - 1:
                pdec_t = work.tile([128, CH, BPC, T], f32, tag="pdec")
                nc.vector.tensor_tensor_scan(
                    out=flat(pdec_t), data0=pseg_t[:], data1=flat(spk_t),
                    initial=0.0, op0=Alu.mult, op1=Alu.add)

                ns_new = small.tile([128, CH, BPC, 1], f32, tag="ns")
                nc.gpsimd.tensor_single_scalar(
                    out=ns_new[:], in_=cnt_t[:, :, :, T - 1:T],
                    scalar=0.0, op=Alu.is_equal)
                vinit_new = small.tile([128, CH, BPC, 1], f32, tag="vinit")
                nc.gpsimd.tensor_tensor(
                    out=vinit_new[:], in0=mem_t[:, :, :, T - 1:T], in1=ns_new[:],
                    op=Alu.mult)
                vb_new = small.tile([128, CH, BPC, 1], f32, tag="vb")
                nc.gpsimd.tensor_tensor(
                    out=vb_new[:], in0=vinit_new[:],
                    in1=betat_t.broadcast_to([128, CH, BPC, 1]), op=Alu.mult)

                a_new = small.tile([128, CH, BPC, 1], f32, tag="a")
                u_t = small.tile([128, CH, BPC, 1], f32, tag="u")
                nc.gpsimd.tensor_tensor(
                    out=u_t[:], in0=pdec_t[:, :, :, T - 1:T],
                    in1=invp_t.broadcast_to([128, CH, BPC, 1]), op=Alu.mult)
                if n == 0:
                    nc.gpsimd.tensor_copy(out=a_new[:], in_=u_t[:])
                else:
                    v_t = small.tile([128, CH, BPC, 1], f32, tag="v")
                    nc.gpsimd.tensor_tensor(
                        out=v_t[:], in0=a_t[:],
                        in1=p32_t.broadcast_to([128, CH, BPC, 1]), op=Alu.mult)
                    nc.gpsimd.tensor_tensor(out=a_new[:], in0=u_t[:], in1=v_t[:],
                                            op=Alu.add)

                thr_new = work.tile([128, CH, BPC, T], f32, tag="thr")
                nc.gpsimd.tensor_tensor(
                    out=thr_new[:], in0=a_new.broadcast_to([128, CH, BPC, T]),
                    in1=bp1_t[:], op=Alu.mult)

                a_t, thr_t, ns_t, vb_t = a_new, thr_new, ns_new, vb_new
            spk_prev, cnt_prev = spk_t, cnt_t

    with tile.TileContext(nc) as tc:
        kern(tc)
    nc.compile()
    return nc


def _host_tables(beta_raw, rec_weight, p_raw, b_raw):
    f = np.float32
    W = rec_weight.astype(f)
    beta = np.clip(beta_raw.astype(f), f(0.001), f(0.999))
    p = np.clip(np.abs(p_raw.astype(f)), f(0.0), f(0.999))
    bb = np.clip(np.abs(b_raw.astype(f)), f(0.001), f(1.0))
    p_pow = (p[:, None] ** np.arange(1, T + 1, dtype=f)).astype(f)   # (C,T)
    BP1 = (bb[:, None] * p_pow).astype(f)
    p32 = np.ascontiguousarray(p_pow[:, -1])
    invp = (f(1.0) / p).astype(f)

    def per_ct(vals_ct):  # (C,T) -> (128, CH*BPC*T), replicated over b
        v = vals_ct.reshape(CH, 128, T)
        out = np.zeros((128, CH, BPC, T), f)
        out[:] = v.transpose(1, 0, 2)[:, :, None, :]
        return np.ascontiguousarray(out.reshape(128, FREE))

    t0mask = np.ones((1, T), f)
    t0mask[0, 0] = 0.0
    betaseg = per_ct((beta[:, None] * t0mask).astype(f))
    pseg = per_ct((p[:, None] * t0mask).astype(f))
    seg01 = per_ct(np.broadcast_to(t0mask, (C, T)).astype(f))
    bp1 = per_ct(BP1)

    def per_c(vals_c):  # (C,) -> (128, CH)
        return np.ascontiguousarray(vals_c.reshape(CH, 128).T)

    # wt[cj_hi*CH + ci_hi][cj_lo, ci_lo] = W[ci_hi*128+ci_lo, cj_hi*128+cj_lo]
    import ml_dtypes
    W4 = W.reshape(CH, 128, CH, 128)
    wt16 = np.ascontiguousarray(
        W4.transpose(2, 0, 3, 1).reshape(16, 128, 128))
    # exact 3-way bf16 decomposition: w1+w2+w3 == W to ~2^-27 relative
    w1 = wt16.astype(ml_dtypes.bfloat16)
    r1 = wt16 - w1.astype(f)
    w2 = r1.astype(ml_dtypes.bfloat16)
    r2 = r1 - w2.astype(f)
    w3 = r2.astype(ml_dtypes.bfloat16)
    wt = np.ascontiguousarray(np.concatenate([w1, w2, w3], axis=0))
    ident = np.eye(128, dtype=f)
    return dict(wt=wt, betat=per_c(beta), ident=ident, betaseg=betaseg,
                pseg=pseg, seg01=seg01, bp1=bp1, p32t=per_c(p32),
                invpt=per_c(invp))


def kernel(x, beta_raw, rec_weight, p_raw, b_raw):
    global _compiled
    from concourse.bass_utils import run_bass_kernel_spmd

    if _compiled is None:
        _compiled = _build_program()
    nc = _compiled

    tables = _host_tables(np.asarray(beta_raw), np.asarray(rec_weight),
                          np.asarray(p_raw), np.asarray(b_raw))
    x = np.ascontiguousarray(np.asarray(x).astype(np.float32))
    in_maps = []
    for k in range(NCORES):
        m = {"x_sh": np.ascontiguousarray(x[k * BPC:(k + 1) * BPC])}
        m.update(tables)
        in_maps.append(m)
    res = run_bass_kernel_spmd(nc, in_maps, list(range(NCORES)))
    out = np.concatenate([res.results[k]["out"] for k in range(NCORES)], axis=0)
    return out.astype(np.float32)


# revision 26
# speedup vs baseline: 1.1730x; 1.1730x over previous
"""Trainium2 Bass kernel for the blockwise spiking network (nn_Blocks_86096914416140).

Sharding: data-parallel over batch B=32 across 8 NeuronCores (4 batches/core),
all parameters replicated, zero collectives.

Per-core device algorithm (fp32 throughout; channel-on-partition layout:
partition = c % 128, tiles [128, (c_hi=4, b=4, t=32)]):

  block n:   xr   = x_blk + W @ spikes_prev + beta*v_init@t0         [TensorE]
             q    = cnt_prev + (1 - spiked_prev)                      [GpSimd]
             cur  = min(q, 1) * xr          (refractory gate)         [VectorE]
             mem  = scan: s = beta*s + cur   (seg-reset via pattern)  [VectorE]
             fs   = (mem - 1) > thr          thr = a*(b*p^{t+1})      [VectorE]
             cnt  = seg-cumsum(fs)                                    [VectorE]
             spk  = (cnt == 1) * fs          (bf16, exact 0/1)        [VectorE]
             pdec = scan: s = p*s + spk  -> last col = p^{31-t0}      [VectorE]
             a'   = p^32 * a + (1/p) * pdec_last                      [GpSimd]
             thr' = a' * (b*p^{t+1})                                  [GpSimd]
             spikes regrouped to 8-block chunks for wide DMA          [ScalarE]

The recurrent weight streams through the PE as an exact 3-way bf16 split
(w1+w2+w3 == W to ~2^-27; spike operand is exactly representable in bf16, so
every product is exact and PSUM accumulates in fp32) — 4x the fp32 matmul
streaming rate with fp32-equivalent rounding. The fp32 x tile is added via an
identity matmul into the same PSUM accumulation group; v_init (pre-scaled by
beta on GpSimd) is injected into the t=0 columns the same way.

All tables are precomputed on the host in fp32 to match the reference's
rounding. Validated bit-exact (zero spike flips) against the fp32 reference
both in CoreSim and on the 8 hardware NeuronCores.
"""

import numpy as np

B, C, T_LEN, T = 32, 512, 1024, 32
NB = T_LEN // T          # 32 blocks
NCORES = 8
BPC = B // NCORES        # 4 batches per core
CH = C // 128            # 4 channel tiles
FREE = CH * BPC * T      # 512 free elements per tile
GRP = 8                  # blocks per x-load group

_compiled = None


def _build_program():
    import concourse.bass as bass
    import concourse.bacc as bacc
    import concourse.tile as tile
    from concourse import mybir
    from concourse._compat import with_exitstack
    from contextlib import ExitStack

    f32 = mybir.dt.float32
    Alu = mybir.AluOpType

    bf16 = mybir.dt.bfloat16

    nc = bacc.Bacc()
    x_d = nc.declare_dram_parameter("x_sh", [BPC, C, T_LEN], f32, isOutput=False)
    wt_d = nc.declare_dram_parameter("wt", [48, 128, 128], bf16, isOutput=False)
    betat_d = nc.declare_dram_parameter("betat", [128, CH], f32, isOutput=False)
    ident_d = nc.declare_dram_parameter("ident", [128, 128], f32, isOutput=False)
    betaseg_d = nc.declare_dram_parameter("betaseg", [128, FREE], f32, isOutput=False)
    pseg_d = nc.declare_dram_parameter("pseg", [128, FREE], f32, isOutput=False)
    seg01_d = nc.declare_dram_parameter("seg01", [128, FREE], f32, isOutput=False)
    bp1_d = nc.declare_dram_parameter("bp1", [128, FREE], f32, isOutput=False)
    p32_d = nc.declare_dram_parameter("p32t", [128, CH], f32, isOutput=False)
    invp_d = nc.declare_dram_parameter("invpt", [128, CH], f32, isOutput=False)
    out_d = nc.declare_dram_parameter("out", [BPC, C, T_LEN], f32, isOutput=True)

    def dram_block_ap(handle, ci, nblk, nt):
        # [128 part = c_lo, (b, t)] view of dram[b, ci*128:(ci+1)*128, nblk*T:...]
        return bass.AP(
            tensor=handle,
            offset=ci * 128 * T_LEN + nblk * T,
            ap=[[T_LEN, 128], [C * T_LEN, BPC], [1, nt]],
        )

    def flat(ap4):
        return ap4.rearrange("p c b t -> p (c b t)")

    @with_exitstack
    def kern(ctx: ExitStack, tc: tile.TileContext):
        consts = ctx.enter_context(tc.tile_pool(name="consts", bufs=1))
        xpool = ctx.enter_context(tc.tile_pool(name="xpool", bufs=2))
        work = ctx.enter_context(tc.tile_pool(name="work", bufs=2))
        spkp = ctx.enter_context(tc.tile_pool(name="spkp", bufs=3))
        small = ctx.enter_context(tc.tile_pool(name="small", bufs=3))
        psum = ctx.enter_context(tc.tile_pool(name="psum", bufs=2, space="PSUM"))

        dma = nc.sync

        wt_t = consts.tile([128, 48, 128], bf16, tag="wt")
        dma.dma_start(out=wt_t[:], in_=wt_d[:].rearrange("k p m -> p k m"))
        betat_t = consts.tile([128, CH, 1, 1], f32, tag="betat")
        dma.dma_start(out=betat_t[:],
                      in_=betat_d[:].rearrange("p (c u v) -> p c u v", u=1, v=1))
        id_t = consts.tile([128, 128], f32, tag="ident")
        dma.dma_start(out=id_t[:], in_=ident_d[:])
        betaseg_t = consts.tile([128, FREE], f32, tag="betaseg")
        dma.dma_start(out=betaseg_t[:], in_=betaseg_d[:])
        pseg_t = consts.tile([128, FREE], f32, tag="pseg")
        dma.dma_start(out=pseg_t[:], in_=pseg_d[:])
        seg01_t = consts.tile([128, FREE], f32, tag="seg01")
        dma.dma_start(out=seg01_t[:], in_=seg01_d[:])
        bp1_t = consts.tile([128, CH, BPC, T], f32, tag="bp1")
        dma.dma_start(out=bp1_t[:],
                      in_=bp1_d[:].rearrange("p (c b t) -> p c b t", c=CH, b=BPC))
        p32_t = consts.tile([128, CH, 1, 1], f32, tag="p32t")
        dma.dma_start(out=p32_t[:],
                      in_=p32_d[:].rearrange("p (c u v) -> p c u v", u=1, v=1))
        invp_t = consts.tile([128, CH, 1, 1], f32, tag="invpt")
        dma.dma_start(out=invp_t[:],
                      in_=invp_d[:].rearrange("p (c u v) -> p c u v", u=1, v=1))

        thr0_t = consts.tile([128, CH, BPC, T], f32, tag="thr0")
        nc.vector.memset(thr0_t[:], 0.0)

        import os
        for _rep in range(int(os.environ.get("BENCH_REPEAT", "1"))):
            _run_once(nc, tc, locals())

    def _run_once(nc, tc, env):
        consts = env["consts"]; xpool = env["xpool"]; work = env["work"]
        spkp = env["spkp"]; small = env["small"]; psum = env["psum"]
        dma = env["dma"]
        wt_t = env["wt_t"]; betat_t = env["betat_t"]; id_t = env["id_t"]
        betaseg_t = env["betaseg_t"]; pseg_t = env["pseg_t"]
        seg01_t = env["seg01_t"]; bp1_t = env["bp1_t"]
        p32_t = env["p32_t"]; invp_t = env["invp_t"]; thr0_t = env["thr0_t"]

        # persistent state (assigned per block)
        a_t = thr_t = vb_t = ns_t = spk_prev = cnt_prev = None

        x_g = None
        sgrp = None
        for n in range(NB):
            gi, go = divmod(n, GRP)
            if go == 0:
                x_g = xpool.tile([128, CH, BPC, GRP * T], f32, tag="xg")
                for ci in range(CH):
                    dma.dma_start(out=x_g[:, ci],
                                  in_=dram_block_ap(x_d, ci, gi * GRP, GRP * T))
                sgrp = xpool.tile([128, CH, BPC, GRP, T], f32, tag="sgrp")

            # channel-half pipelining: half B's matmuls overlap half A's
            # VectorE chain (the inter-block spike->matmul chain only gates
            # per-half, halving the serial stall).
            cur_t = work.tile([128, CH, BPC, T], f32, tag="cur")
            mem_t = work.tile([128, CH, BPC, T], f32, tag="mem")
            fs_t = work.tile([128, CH, BPC, T], f32, tag="fs")
            cnt_t = work.tile([128, CH, BPC, T], f32, tag="cnt")
            spk_t = spkp.tile([128, CH, BPC, T], bf16, tag="spk")
            HH = CH // 2
            HF = HH * BPC * T

            def hsl(t4, h):  # contiguous channel-half slice, flattened
                return t4[:, h * HH:(h + 1) * HH].rearrange("p c b t -> p (c b t)")

            xrh = [None, None]
            q_t = None
            if n > 0:
                q_t = work.tile([128, CH, BPC, T], f32, tag="gate")
                nc.gpsimd.tensor_tensor(
                    out=q_t[:], in0=cnt_prev[:],
                    in1=ns_t.broadcast_to([128, CH, BPC, T]), op=Alu.add)
                for h in range(2):
                    xr = psum.tile([128, HH, BPC, T], f32, tag=f"xr{h}")
                    xrh[h] = xr
                    nc.tensor.matmul(
                        out=xr[:], lhsT=id_t[:],
                        rhs=x_g[:, h * HH:(h + 1) * HH, :, go * T:(go + 1) * T],
                        start=True, stop=False)
                    for v in range(3):
                        for cl in range(HH):
                            ci = h * HH + cl
                            for cj in range(CH):
                                nc.tensor.matmul(
                                    out=xr[:, cl],
                                    lhsT=wt_t[:, v * 16 + cj * CH + ci],
                                    rhs=spk_prev[:, cj],
                                    start=False, stop=False)
                    # v_init (pre-scaled by beta on GpSimd) into t=0 columns
                    nc.tensor.matmul(
                        out=xr[:, :, :, 0:1], lhsT=id_t[:],
                        rhs=vb_t[:, h * HH:(h + 1) * HH].rearrange(
                            "p c b u -> p (c b u)"),
                        start=False, stop=True)

            for h in range(2):
                if n == 0:
                    nc.vector.tensor_copy(
                        out=cur_t[:, h * HH:(h + 1) * HH],
                        in_=x_g[:, h * HH:(h + 1) * HH, :, go * T:(go + 1) * T])
                    thr_t = thr0_t
                else:
                    # refractory gate: cur = min(cnt_prev + (1-mf), 1) * xr
                    nc.vector.scalar_tensor_tensor(
                        out=hsl(cur_t, h), in0=hsl(q_t, h), scalar=1.0,
                        in1=xrh[h].rearrange("p c b t -> p (c b t)"),
                        op0=Alu.min, op1=Alu.mult)

                nc.vector.tensor_tensor_scan(
                    out=hsl(mem_t, h), data0=betaseg_t[:, h * HF:(h + 1) * HF],
                    data1=hsl(cur_t, h),
                    initial=0.0, op0=Alu.mult, op1=Alu.add)

                nc.vector.scalar_tensor_tensor(
                    out=hsl(fs_t, h), in0=hsl(mem_t, h), scalar=1.0,
                    in1=hsl(thr_t, h), op0=Alu.subtract, op1=Alu.is_gt)

                nc.vector.tensor_tensor_scan(
                    out=hsl(cnt_t, h), data0=seg01_t[:, h * HF:(h + 1) * HF],
                    data1=hsl(fs_t, h),
                    initial=0.0, op0=Alu.mult, op1=Alu.add)

                nc.vector.scalar_tensor_tensor(
                    out=hsl(spk_t, h), in0=hsl(cnt_t, h), scalar=1.0,
                    in1=hsl(fs_t, h), op0=Alu.is_equal, op1=Alu.mult)

            nc.scalar.copy(out=sgrp[:, :, :, go], in_=spk_t[:])
            if go == GRP - 1:
                for ci in range(CH):
                    dma.dma_start(
                        out=dram_block_ap(out_d, ci, gi * GRP, GRP * T),
                        in_=sgrp[:, ci])

            if n < NB - 1:
                pdec_t = work.tile([128, CH, BPC, T], f32, tag="pdec")
                nc.vector.tensor_tensor_scan(
                    out=flat(pdec_t), data0=pseg_t[:], data1=flat(spk_t),
                    initial=0.0, op0=Alu.mult, op1=Alu.add)

                ns_new = small.tile([128, CH, BPC, 1], f32, tag="ns")
                nc.gpsimd.tensor_single_scalar(
                    out=ns_new[:], in_=cnt_t[:, :, :, T - 1:T],
                    scalar=0.0, op=Alu.is_equal)
                vinit_new = small.tile([128, CH, BPC, 1], f32, tag="vinit")
                nc.gpsimd.tensor_tensor(
                    out=vinit_new[:], in0=mem_t[:, :, :, T - 1:T], in1=ns_new[:],
                    op=Alu.mult)
                vb_new = small.tile([128, CH, BPC, 1], f32, tag="vb")
                nc.gpsimd.tensor_tensor(
                    out=vb_new[:], in0=vinit_new[:],
                    in1=betat_t.broadcast_to([128, CH, BPC, 1]), op=Alu.mult)

                a_new = small.tile([128, CH, BPC, 1], f32, tag="a")
                u_t = small.tile([128, CH, BPC, 1], f32, tag="u")
                nc.gpsimd.tensor_tensor(
                    out=u_t[:], in0=pdec_t[:, :, :, T - 1:T],
                    in1=invp_t.broadcast_to([128, CH, BPC, 1]), op=Alu.mult)
                if n == 0:
                    nc.gpsimd.tensor_copy(out=a_new[:], in_=u_t[:])
                else:
                    v_t = small.tile([128, CH, BPC, 1], f32, tag="v")
                    nc.gpsimd.tensor_tensor(
                        out=v_t[:], in0=a_t[:],
                        in1=p32_t.broadcast_to([128, CH, BPC, 1]), op=Alu.mult)
                    nc.gpsimd.tensor_tensor(out=a_new[:], in0=u_t[:], in1=v_t[:],
                                            op=Alu.add)

                thr_new = work.tile([128, CH, BPC, T], f32, tag="thr")
                nc.gpsimd.tensor_tensor(
                    out=thr_new[:], in0=a_new.broadcast_to([128, CH, BPC, T]),
                    in1=bp1_t[:], op=Alu.mult)

                a_t, thr_t, ns_t, vb_t = a_new, thr_new, ns_new, vb_new
            spk_prev, cnt_prev = spk_t, cnt_t

    with tile.TileContext(nc) as tc:
        kern(tc)
    nc.compile()
    return nc


def _host_tables(beta_raw, rec_weight, p_raw, b_raw):
    f = np.float32
    W = rec_weight.astype(f)
    beta = np.clip(beta_raw.astype(f), f(0.001), f(0.999))
    p = np.clip(np.abs(p_raw.astype(f)), f(0.0), f(0.999))
    bb = np.clip(np.abs(b_raw.astype(f)), f(0.001), f(1.0))
    p_pow = (p[:, None] ** np.arange(1, T + 1, dtype=f)).astype(f)   # (C,T)
    BP1 = (bb[:, None] * p_pow).astype(f)
    p32 = np.ascontiguousarray(p_pow[:, -1])
    invp = (f(1.0) / p).astype(f)

    def per_ct(vals_ct):  # (C,T) -> (128, CH*BPC*T), replicated over b
        v = vals_ct.reshape(CH, 128, T)
        out = np.zeros((128, CH, BPC, T), f)
        out[:] = v.transpose(1, 0, 2)[:, :, None, :]
        return np.ascontiguousarray(out.reshape(128, FREE))

    t0mask = np.ones((1, T), f)
    t0mask[0, 0] = 0.0
    betaseg = per_ct((beta[:, None] * t0mask).astype(f))
    pseg = per_ct((p[:, None] * t0mask).astype(f))
    seg01 = per_ct(np.broadcast_to(t0mask, (C, T)).astype(f))
    bp1 = per_ct(BP1)

    def per_c(vals_c):  # (C,) -> (128, CH)
        return np.ascontiguousarray(vals_c.reshape(CH, 128).T)

    # wt[cj_hi*CH + ci_hi][cj_lo, ci_lo] = W[ci_hi*128+ci_lo, cj_hi*128+cj_lo]
    import ml_dtypes
    W4 = W.reshape(CH, 128, CH, 128)
    wt16 = np.ascontiguousarray(
        W4.transpose(2, 0, 3, 1).reshape(16, 128, 128))
    # exact 3-way bf16 decomposition: w1+w2+w3 == W to ~2^-27 relative
    w1 = wt16.astype(ml_dtypes.bfloat16)
    r1 = wt16 - w1.astype(f)
    w2 = r1.astype(ml_dtypes.bfloat16)
    r2 = r1 - w2.astype(f)
    w3 = r2.astype(ml_dtypes.bfloat16)
    wt = np.ascontiguousarray(np.concatenate([w1, w2, w3], axis=0))
    ident = np.eye(128, dtype=f)
    return dict(wt=wt, betat=per_c(beta), ident=ident, betaseg=betaseg,
                pseg=pseg, seg01=seg01, bp1=bp1, p32t=per_c(p32),
                invpt=per_c(invp))


def kernel(x, beta_raw, rec_weight, p_raw, b_raw):
    global _compiled
    from concourse.bass_utils import run_bass_kernel_spmd

    if _compiled is None:
        _compiled = _build_program()
    nc = _compiled

    tables = _host_tables(np.asarray(beta_raw), np.asarray(rec_weight),
                          np.asarray(p_raw), np.asarray(b_raw))
    x = np.ascontiguousarray(np.asarray(x).astype(np.float32))
    in_maps = []
    for k in range(NCORES):
        m = {"x_sh": np.ascontiguousarray(x[k * BPC:(k + 1) * BPC])}
        m.update(tables)
        in_maps.append(m)
    res = run_bass_kernel_spmd(nc, in_maps, list(range(NCORES)))
    out = np.concatenate([res.results[k]["out"] for k in range(NCORES)], axis=0)
    return out.astype(np.float32)


# revision 28
# speedup vs baseline: 1.2373x; 1.0548x over previous
"""Trainium2 Bass kernel for the blockwise spiking network (nn_Blocks_86096914416140).

Sharding: data-parallel over batch B=32 across 8 NeuronCores (4 batches/core),
all parameters replicated, zero collectives.

Per-core device algorithm (fp32 throughout; channel-on-partition layout:
partition = c % 128, tiles [128, (c_hi=4, b=4, t=32)]):

  block n:   xr   = x_blk + W @ spikes_prev + beta*v_init@t0         [TensorE]
             q    = cnt_prev + (1 - spiked_prev)                      [GpSimd]
             cur  = min(q, 1) * xr          (refractory gate)         [VectorE]
             mem  = scan: s = beta*s + cur   (seg-reset via pattern)  [VectorE]
             fs   = (mem - 1) > thr          thr = a*(b*p^{t+1})      [VectorE]
             cnt  = seg-cumsum(fs)                                    [VectorE]
             spk  = (cnt == 1) * fs          (bf16, exact 0/1)        [VectorE]
             pdec = scan: s = p*s + spk  -> last col = p^{31-t0}      [VectorE]
             a'   = p^32 * a + (1/p) * pdec_last                      [GpSimd]
             thr' = a' * (b*p^{t+1})                                  [GpSimd]
             spikes regrouped to 8-block chunks for wide DMA          [ScalarE]

The recurrent weight streams through the PE as an exact 3-way bf16 split
(w1+w2+w3 == W to ~2^-27; spike operand is exactly representable in bf16, so
every product is exact and PSUM accumulates in fp32) — 4x the fp32 matmul
streaming rate with fp32-equivalent rounding. The fp32 x tile is added via an
identity matmul into the same PSUM accumulation group; v_init (pre-scaled by
beta on GpSimd) is injected into the t=0 columns the same way.

All tables are precomputed on the host in fp32 to match the reference's
rounding. Validated bit-exact (zero spike flips) against the fp32 reference
both in CoreSim and on the 8 hardware NeuronCores.
"""

import numpy as np

B, C, T_LEN, T = 32, 512, 1024, 32
NB = T_LEN // T          # 32 blocks
NCORES = 8
BPC = B // NCORES        # 4 batches per core
CH = C // 128            # 4 channel tiles
FREE = CH * BPC * T      # 512 free elements per tile
GRP = 8                  # blocks per x-load group

_compiled = None


def _build_program():
    import concourse.bass as bass
    import concourse.bacc as bacc
    import concourse.tile as tile
    from concourse import mybir
    from concourse._compat import with_exitstack
    from contextlib import ExitStack

    f32 = mybir.dt.float32
    Alu = mybir.AluOpType

    bf16 = mybir.dt.bfloat16

    nc = bacc.Bacc()
    x_d = nc.declare_dram_parameter("x_sh", [BPC, C, T_LEN], f32, isOutput=False)
    wt_d = nc.declare_dram_parameter("wt", [48, 128, 128], bf16, isOutput=False)
    betat_d = nc.declare_dram_parameter("betat", [128, CH], f32, isOutput=False)
    ident_d = nc.declare_dram_parameter("ident", [128, 128], f32, isOutput=False)
    betaseg_d = nc.declare_dram_parameter("betaseg", [128, FREE], f32, isOutput=False)
    pseg_d = nc.declare_dram_parameter("pseg", [128, FREE], f32, isOutput=False)
    seg01_d = nc.declare_dram_parameter("seg01", [128, FREE], f32, isOutput=False)
    bp1_d = nc.declare_dram_parameter("bp1", [128, FREE], f32, isOutput=False)
    p32_d = nc.declare_dram_parameter("p32t", [128, CH], f32, isOutput=False)
    invp_d = nc.declare_dram_parameter("invpt", [128, CH], f32, isOutput=False)
    out_d = nc.declare_dram_parameter("out", [BPC, C, T_LEN], f32, isOutput=True)

    def dram_block_ap(handle, ci, nblk, nt):
        # [128 part = c_lo, (b, t)] view of dram[b, ci*128:(ci+1)*128, nblk*T:...]
        return bass.AP(
            tensor=handle,
            offset=ci * 128 * T_LEN + nblk * T,
            ap=[[T_LEN, 128], [C * T_LEN, BPC], [1, nt]],
        )

    def flat(ap4):
        return ap4.rearrange("p c b t -> p (c b t)")

    @with_exitstack
    def kern(ctx: ExitStack, tc: tile.TileContext):
        consts = ctx.enter_context(tc.tile_pool(name="consts", bufs=1))
        xpool = ctx.enter_context(tc.tile_pool(name="xpool", bufs=2))
        work = ctx.enter_context(tc.tile_pool(name="work", bufs=2))
        spkp = ctx.enter_context(tc.tile_pool(name="spkp", bufs=3))
        small = ctx.enter_context(tc.tile_pool(name="small", bufs=3))
        psum = ctx.enter_context(tc.tile_pool(name="psum", bufs=2, space="PSUM"))

        dma = nc.sync

        wt_t = consts.tile([128, 48, 128], bf16, tag="wt")
        dma.dma_start(out=wt_t[:], in_=wt_d[:].rearrange("k p m -> p k m"))
        betat_t = consts.tile([128, CH, 1, 1], f32, tag="betat")
        dma.dma_start(out=betat_t[:],
                      in_=betat_d[:].rearrange("p (c u v) -> p c u v", u=1, v=1))
        id_t = consts.tile([128, 128], f32, tag="ident")
        dma.dma_start(out=id_t[:], in_=ident_d[:])
        betaseg_t = consts.tile([128, FREE], f32, tag="betaseg")
        dma.dma_start(out=betaseg_t[:], in_=betaseg_d[:])
        pseg_t = consts.tile([128, FREE], f32, tag="pseg")
        dma.dma_start(out=pseg_t[:], in_=pseg_d[:])
        seg01_t = consts.tile([128, FREE], f32, tag="seg01")
        dma.dma_start(out=seg01_t[:], in_=seg01_d[:])
        bp1_t = consts.tile([128, CH, BPC, T], f32, tag="bp1")
        dma.dma_start(out=bp1_t[:],
                      in_=bp1_d[:].rearrange("p (c b t) -> p c b t", c=CH, b=BPC))
        p32_t = consts.tile([128, CH, 1, 1], f32, tag="p32t")
        dma.dma_start(out=p32_t[:],
                      in_=p32_d[:].rearrange("p (c u v) -> p c u v", u=1, v=1))
        invp_t = consts.tile([128, CH, 1, 1], f32, tag="invpt")
        dma.dma_start(out=invp_t[:],
                      in_=invp_d[:].rearrange("p (c u v) -> p c u v", u=1, v=1))

        thr0_t = consts.tile([128, CH, BPC, T], f32, tag="thr0")
        nc.vector.memset(thr0_t[:], 0.0)

        import os
        for _rep in range(int(os.environ.get("BENCH_REPEAT", "1"))):
            _run_once(nc, tc, locals())

    def _run_once(nc, tc, env):
        consts = env["consts"]; xpool = env["xpool"]; work = env["work"]
        spkp = env["spkp"]; small = env["small"]; psum = env["psum"]
        dma = env["dma"]
        wt_t = env["wt_t"]; betat_t = env["betat_t"]; id_t = env["id_t"]
        betaseg_t = env["betaseg_t"]; pseg_t = env["pseg_t"]
        seg01_t = env["seg01_t"]; bp1_t = env["bp1_t"]
        p32_t = env["p32_t"]; invp_t = env["invp_t"]; thr0_t = env["thr0_t"]

        # persistent state (assigned per block)
        a_t = thr_t = vb_t = ns_t = spk_prev = cnt_prev = None

        x_g = None
        sgrp = None
        for n in range(NB):
            gi, go = divmod(n, GRP)
            if go == 0:
                x_g = xpool.tile([128, CH, BPC, GRP * T], f32, tag="xg")
                for ci in range(CH):
                    dma.dma_start(out=x_g[:, ci],
                                  in_=dram_block_ap(x_d, ci, gi * GRP, GRP * T))
                sgrp = xpool.tile([128, CH, BPC, GRP, T], f32, tag="sgrp")

            # channel-half pipelining: half B's matmuls overlap half A's
            # VectorE chain (the inter-block spike->matmul chain only gates
            # per-half, halving the serial stall).
            cur_t = work.tile([128, CH, BPC, T], f32, tag="cur")
            mem_t = work.tile([128, CH, BPC, T], f32, tag="mem")
            fs_t = work.tile([128, CH, BPC, T], f32, tag="fs")
            cnt_t = work.tile([128, CH, BPC, T], f32, tag="cnt")
            spk_t = spkp.tile([128, CH, BPC, T], bf16, tag="spk")
            HH = CH // 2
            HF = HH * BPC * T

            def hsl(t4, h):  # contiguous channel-half slice, flattened
                return t4[:, h * HH:(h + 1) * HH].rearrange("p c b t -> p (c b t)")

            xrh = [None, None]
            q_t = None
            if n > 0:
                q_t = work.tile([128, CH, BPC, T], f32, tag="gate")
                nc.gpsimd.tensor_tensor(
                    out=q_t[:], in0=cnt_prev[:],
                    in1=ns_t.broadcast_to([128, CH, BPC, T]), op=Alu.add)
                for h in range(2):
                    xr = psum.tile([128, HH, BPC, T], f32, tag=f"xr{h}")
                    xrh[h] = xr
                    nc.tensor.matmul(
                        out=xr[:], lhsT=id_t[:],
                        rhs=x_g[:, h * HH:(h + 1) * HH, :, go * T:(go + 1) * T],
                        start=True, stop=False)
                # order the contraction by spike-readiness: cj in {0,1} needs
                # only the previous block's half-A spikes, so those matmuls
                # overlap half-B's VectorE chain; only the cj {2,3} quarter
                # remains exposed after half-B's spikes land.
                def wmms(h, cjs):
                    for v in range(3):
                        for cl in range(HH):
                            ci = h * HH + cl
                            for cj in cjs:
                                nc.tensor.matmul(
                                    out=xrh[h][:, cl],
                                    lhsT=wt_t[:, v * 16 + cj * CH + ci],
                                    rhs=spk_prev[:, cj],
                                    start=False, stop=False)

                wmms(0, (0, 1))
                wmms(1, (0, 1))
                for h in range(2):
                    wmms(h, (2, 3))
                    # v_init (pre-scaled by beta on GpSimd) into t=0 columns
                    nc.tensor.matmul(
                        out=xrh[h][:, :, :, 0:1], lhsT=id_t[:],
                        rhs=vb_t[:, h * HH:(h + 1) * HH].rearrange(
                            "p c b u -> p (c b u)"),
                        start=False, stop=True)

            for h in range(2):
                if n == 0:
                    nc.vector.tensor_copy(
                        out=cur_t[:, h * HH:(h + 1) * HH],
                        in_=x_g[:, h * HH:(h + 1) * HH, :, go * T:(go + 1) * T])
                    thr_t = thr0_t
                else:
                    # refractory gate: cur = min(cnt_prev + (1-mf), 1) * xr
                    nc.vector.scalar_tensor_tensor(
                        out=hsl(cur_t, h), in0=hsl(q_t, h), scalar=1.0,
                        in1=xrh[h].rearrange("p c b t -> p (c b t)"),
                        op0=Alu.min, op1=Alu.mult)

                nc.vector.tensor_tensor_scan(
                    out=hsl(mem_t, h), data0=betaseg_t[:, h * HF:(h + 1) * HF],
                    data1=hsl(cur_t, h),
                    initial=0.0, op0=Alu.mult, op1=Alu.add)

                nc.vector.scalar_tensor_tensor(
                    out=hsl(fs_t, h), in0=hsl(mem_t, h), scalar=1.0,
                    in1=hsl(thr_t, h), op0=Alu.subtract, op1=Alu.is_gt)

                nc.vector.tensor_tensor_scan(
                    out=hsl(cnt_t, h), data0=seg01_t[:, h * HF:(h + 1) * HF],
                    data1=hsl(fs_t, h),
                    initial=0.0, op0=Alu.mult, op1=Alu.add)

                nc.vector.scalar_tensor_tensor(
                    out=hsl(spk_t, h), in0=hsl(cnt_t, h), scalar=1.0,
                    in1=hsl(fs_t, h), op0=Alu.is_equal, op1=Alu.mult)

            nc.scalar.copy(out=sgrp[:, :, :, go], in_=spk_t[:])
            if go == GRP - 1:
                for ci in range(CH):
                    dma.dma_start(
                        out=dram_block_ap(out_d, ci, gi * GRP, GRP * T),
                        in_=sgrp[:, ci])

            if n < NB - 1:
                pdec_t = work.tile([128, CH, BPC, T], f32, tag="pdec")
                nc.vector.tensor_tensor_scan(
                    out=flat(pdec_t), data0=pseg_t[:], data1=flat(spk_t),
                    initial=0.0, op0=Alu.mult, op1=Alu.add)

                ns_new = small.tile([128, CH, BPC, 1], f32, tag="ns")
                nc.gpsimd.tensor_single_scalar(
                    out=ns_new[:], in_=cnt_t[:, :, :, T - 1:T],
                    scalar=0.0, op=Alu.is_equal)
                vinit_new = small.tile([128, CH, BPC, 1], f32, tag="vinit")
                nc.gpsimd.tensor_tensor(
                    out=vinit_new[:], in0=mem_t[:, :, :, T - 1:T], in1=ns_new[:],
                    op=Alu.mult)
                vb_new = small.tile([128, CH, BPC, 1], f32, tag="vb")
                nc.gpsimd.tensor_tensor(
                    out=vb_new[:], in0=vinit_new[:],
                    in1=betat_t.broadcast_to([128, CH, BPC, 1]), op=Alu.mult)

                a_new = small.tile([128, CH, BPC, 1], f32, tag="a")
                u_t = small.tile([128, CH, BPC, 1], f32, tag="u")
                nc.gpsimd.tensor_tensor(
                    out=u_t[:], in0=pdec_t[:, :, :, T - 1:T],
                    in1=invp_t.broadcast_to([128, CH, BPC, 1]), op=Alu.mult)
                if n == 0:
                    nc.gpsimd.tensor_copy(out=a_new[:], in_=u_t[:])
                else:
                    v_t = small.tile([128, CH, BPC, 1], f32, tag="v")
                    nc.gpsimd.tensor_tensor(
                        out=v_t[:], in0=a_t[:],
                        in1=p32_t.broadcast_to([128, CH, BPC, 1]), op=Alu.mult)
                    nc.gpsimd.tensor_tensor(out=a_new[:], in0=u_t[:], in1=v_t[:],
                                            op=Alu.add)

                thr_new = work.tile([128, CH, BPC, T], f32, tag="thr")
                nc.gpsimd.tensor_tensor(
                    out=thr_new[:], in0=a_new.broadcast_to([128, CH, BPC, T]),
                    in1=bp1_t[:], op=Alu.mult)

                a_t, thr_t, ns_t, vb_t = a_new, thr_new, ns_new, vb_new
            spk_prev, cnt_prev = spk_t, cnt_t

    with tile.TileContext(nc) as tc:
        kern(tc)
    nc.compile()
    return nc


def _host_tables(beta_raw, rec_weight, p_raw, b_raw):
    f = np.float32
    W = rec_weight.astype(f)
    beta = np.clip(beta_raw.astype(f), f(0.001), f(0.999))
    p = np.clip(np.abs(p_raw.astype(f)), f(0.0), f(0.999))
    bb = np.clip(np.abs(b_raw.astype(f)), f(0.001), f(1.0))
    p_pow = (p[:, None] ** np.arange(1, T + 1, dtype=f)).astype(f)   # (C,T)
    BP1 = (bb[:, None] * p_pow).astype(f)
    p32 = np.ascontiguousarray(p_pow[:, -1])
    invp = (f(1.0) / p).astype(f)

    def per_ct(vals_ct):  # (C,T) -> (128, CH*BPC*T), replicated over b
        v = vals_ct.reshape(CH, 128, T)
        out = np.zeros((128, CH, BPC, T), f)
        out[:] = v.transpose(1, 0, 2)[:, :, None, :]
        return np.ascontiguousarray(out.reshape(128, FREE))

    t0mask = np.ones((1, T), f)
    t0mask[0, 0] = 0.0
    betaseg = per_ct((beta[:, None] * t0mask).astype(f))
    pseg = per_ct((p[:, None] * t0mask).astype(f))
    seg01 = per_ct(np.broadcast_to(t0mask, (C, T)).astype(f))
    bp1 = per_ct(BP1)

    def per_c(vals_c):  # (C,) -> (128, CH)
        return np.ascontiguousarray(vals_c.reshape(CH, 128).T)

    # wt[cj_hi*CH + ci_hi][cj_lo, ci_lo] = W[ci_hi*128+ci_lo, cj_hi*128+cj_lo]
    import ml_dtypes
    W4 = W.reshape(CH, 128, CH, 128)
    wt16 = np.ascontiguousarray(
        W4.transpose(2, 0, 3, 1).reshape(16, 128, 128))
    # exact 3-way bf16 decomposition: w1+w2+w3 == W to ~2^-27 relative
    w1 = wt16.astype(ml_dtypes.bfloat16)
    r1 = wt16 - w1.astype(f)
    w2 = r1.astype(ml_dtypes.bfloat16)
    r2 = r1 - w2.astype(f)
    w3 = r2.astype(ml_dtypes.bfloat16)
    wt = np.ascontiguousarray(np.concatenate([w1, w2, w3], axis=0))
    ident = np.eye(128, dtype=f)
    return dict(wt=wt, betat=per_c(beta), ident=ident, betaseg=betaseg,
                pseg=pseg, seg01=seg01, bp1=bp1, p32t=per_c(p32),
                invpt=per_c(invp))


def kernel(x, beta_raw, rec_weight, p_raw, b_raw):
    global _compiled
    from concourse.bass_utils import run_bass_kernel_spmd

    if _compiled is None:
        _compiled = _build_program()
    nc = _compiled

    tables = _host_tables(np.asarray(beta_raw), np.asarray(rec_weight),
                          np.asarray(p_raw), np.asarray(b_raw))
    x = np.ascontiguousarray(np.asarray(x).astype(np.float32))
    in_maps = []
    for k in range(NCORES):
        m = {"x_sh": np.ascontiguousarray(x[k * BPC:(k + 1) * BPC])}
        m.update(tables)
        in_maps.append(m)
    res = run_bass_kernel_spmd(nc, in_maps, list(range(NCORES)))
    out = np.concatenate([res.results[k]["out"] for k in range(NCORES)], axis=0)
    return out.astype(np.float32)
